# revision 28
# baseline (speedup 1.0000x reference)
"""Expert-parallel MoE MLP (ExpertMLP) Bass kernel for 8 Trainium2 NeuronCores.

Problem: x[32,4096,256] @ w_fc[32,256,1024] -> gelu(erf) -> @ w_proj[32,1024,256].

Sharding: expert-parallel, 4 experts per core, no cross-core communication.

Per-core dataflow (v2 — software-pipelined):

  * x[e] is staged to bf16 in DRAM slab-by-slab (1024 rows): HWDGE f32 load
    -> GpSimd cast -> HWDGE bf16 store -> XBar DMA-transpose into SBUF as
    xT[d, c] slabs. v2 used SWDGE DRAM->DRAM cast-DMAs here; their packet
    flood starved the HWDGE queue to ~1% bandwidth and stalled the PE 62us.
  * MM1: hT[h_tile, c] += w_fc[d,h_tile].T @ xT[d, c-chunk]; GELU (ACT) evicts
    PSUM->SBUF in bf16.
  * MM2: out[c_sub, d] += hT_slice.T @ w_proj[h, d], PSUM -> DVE copy -> DMA.
  * The PE instruction stream interleaves [MM2 of chunk g-1] with [MM1 of
    chunk g] so the GELU chain (5.3us per chunk on ACT) has a full chunk of
    slack and never stalls the PE (v1 lost ~2.2us per chunk to it).
  * ~24 dummy matmuls on a zeroed tile run at t~0 to open the HAM clock gate
    (PE idles at 1.2 GHz otherwise) while DMAs stage the first slabs.
  * Weights and x slabs for expert e+1 prefetch across expert e's chunks.

All matmul operands are bf16 (PSUM accumulation stays fp32); measured rel err
vs the f32 reference ~4e-3 (tolerance 2e-2).
"""

import numpy as np
from contextlib import ExitStack

import bass_rust as _br
import concourse.bass as bass
import concourse.tile as tile
from concourse import mybir
from concourse.bass_utils import run_bass_kernel_spmd

E, CAP, D, H = 32, 4096, 256, 1024
N_CORES = 8
E_PER = E // N_CORES  # 4 experts per core
P = 128
F32 = mybir.dt.float32
BF16 = mybir.dt.bfloat16

KD = D // P            # 2 k-tiles in MM1's contraction
KH = H // P            # 8 k-tiles in MM2's contraction
NC_CHUNK = 512         # capacity chunk per pipeline block
CHUNKS_PER_E = CAP // NC_CHUNK   # 8
SLAB = 512             # x staging slab == chunk: one slab per block
SLABS_PER_E = CAP // SLAB        # 8
HPACK = 2              # h_tiles packed per PSUM tile / GELU call
HGROUPS = H // P // HPACK        # 4
G_TOTAL = E_PER * CHUNKS_PER_E   # 32 pipeline blocks of real work


def _fix_waits(nc):
    """walrus here accepts only one sync wait per instruction; hoist excess
    waits onto standalone EventSemaphore instructions inserted before the
    offender (same engine => same sequencer order)."""
    for fn in nc.m.functions:
        for bb in fn.blocks:
            new = []
            changed = False
            for inst in bb.instructions:
                si = inst.sync_info
                if si is not None and len(si.on_wait) > 1:
                    waits = list(si.on_wait)
                    for w in waits[:-1]:
                        ev = mybir.InstEventSemaphore(
                            name=nc.get_next_instruction_name()
                        )
                        ev.engine = inst.engine
                        ev.sync_info = _br.SyncInfo(on_wait=[w], on_update=[])
                        nc.register_instruction(ev)
                        new.append(ev)
                    inst.sync_info = _br.SyncInfo(
                        on_wait=waits[-1:], on_update=list(si.on_update)
                    )
                    changed = True
                new.append(inst)
            if changed:
                bb.instructions = new


def _build():
    nc = bass.Bass(trn_type="TRN2", target_bir_lowering=False, debug=False)
    x = nc.dram_tensor("x", [E_PER, CAP, D], F32, kind="ExternalInput").ap()
    w_fc = nc.dram_tensor("w_fc", [E_PER, D, H], F32, kind="ExternalInput").ap()
    w_proj = nc.dram_tensor("w_proj", [E_PER, H, D], F32, kind="ExternalInput").ap()
    out = nc.dram_tensor("out", [E_PER, CAP, D], F32, kind="ExternalOutput").ap()
    # bf16 staging of x, one DRAM tensor per (expert, slab): DRAM dependency
    # tracking is tensor-granular, so each XBar transpose starts as soon as
    # its own slab's cast lands.
    xbf = [
        [
            nc.dram_tensor(f"xbf{e}_{s}", [SLAB, D], BF16).ap()
            for s in range(SLABS_PER_E)
        ]
        for e in range(E_PER)
    ]

    with tile.TileContext(nc) as tc, ExitStack() as ctx:
        xtp = ctx.enter_context(tc.tile_pool(name="xtp", bufs=2 * SLABS_PER_E * KD))
        xraw_p = ctx.enter_context(tc.tile_pool(name="xraw", bufs=3))
        xbfs_p = ctx.enter_context(tc.tile_pool(name="xbfs", bufs=3))
        wload = ctx.enter_context(tc.tile_pool(name="wload", bufs=2))
        wfc_p = ctx.enter_context(tc.tile_pool(name="wfc", bufs=2))
        wproj_p = ctx.enter_context(tc.tile_pool(name="wproj", bufs=2))
        ht_p = ctx.enter_context(tc.tile_pool(name="ht", bufs=2 * HGROUPS))
        out_p = ctx.enter_context(tc.tile_pool(name="outp", bufs=4))
        misc_p = ctx.enter_context(tc.tile_pool(name="misc", bufs=1))
        ps_h = ctx.enter_context(tc.tile_pool(name="ps_h", bufs=2, space="PSUM"))
        ps_o = ctx.enter_context(tc.tile_pool(name="ps_o", bufs=4, space="PSUM"))

        # ---- PE warmup: the HAM clock gate keeps the PE at 1.2 GHz until it
        # has seen ~3.4us of sustained activity. Dummy matmuls on a zeroed
        # tile warm it while the first slab casts/transposes are in flight.
        wu = misc_p.tile([P, P], BF16, tag="wu")
        nc.vector.memset(wu[:], 0.0)
        wps = ps_o.tile([P, D], F32, tag="pso")
        for _ in range(40):
            nc.tensor.matmul(wps[:, 0:P], wu[:], wu[:], start=True, stop=True)

        xts = [[[None] * SLABS_PER_E for _ in range(KD)] for _ in range(E_PER)]
        SLABW = SLAB // P * D  # 2048 elems/partition per slab tile
        stage = {}  # in-flight x staging tiles: ('r'|'b', e, s) -> tile

        def x_load(e, s):
            rs = slice(s * SLAB, (s + 1) * SLAB)
            raw = xraw_p.tile([P, SLABW], F32, tag="xr", name=f"xr{e}_{s}")
            nc.sync.dma_start(
                raw[:], x[e][rs].rearrange("(p s) d -> p (s d)", p=P)
            )
            stage[("r", e, s)] = raw

        def x_cast(e, s, eng=None):
            # steady-state casts run on the otherwise-idle GpSimd engine:
            # slow (~7us/slab vs 1.3 on DVE) but cadence is one per ~14us,
            # and its FIFO head-blocking can't convoy the copies (DVE) or
            # the DMA issue stream (Sync). Prologue casts use DVE for speed.
            bfv = xbfs_p.tile([P, SLABW], BF16, tag="xb", name=f"xb{e}_{s}")
            (eng or nc.gpsimd).tensor_copy(bfv[:], stage.pop(("r", e, s))[:])
            stage[("b", e, s)] = bfv

        def x_store(e, s):
            nc.sync.dma_start(
                xbf[e][s].rearrange("(p s) d -> p (s d)", p=P),
                stage.pop(("b", e, s))[:],
            )

        def issue_transpose(e, s):
            for k in range(KD):
                t = xtp.tile([P, SLAB], BF16, tag="xt", name=f"xt{e}_{k}_{s}")
                nc.sync.dma_start_transpose(t[:], xbf[e][s][:, k * P:(k + 1) * P])
                xts[e][k][s] = t

        def load_wfc_raw(e):
            raw = wload.tile([P, KD, H], F32, tag="wl")
            nc.sync.dma_start(raw[:], w_fc[e].rearrange("(k p) h -> p k h", p=P))
            return raw

        def cast_wfc(raw, eng=None):
            t = wfc_p.tile([P, KD, H], BF16, tag="wfc")
            (eng or nc.gpsimd).tensor_copy(t[:], raw[:])
            return t

        def load_wproj_raw(e):
            raw = wload.tile([P, KH, D], F32, tag="wl")
            nc.sync.dma_start(raw[:], w_proj[e].rearrange("(k p) d -> p k d", p=P))
            return raw

        def cast_wproj(raw, eng=None):
            t = wproj_p.tile([P, KH, D], BF16, tag="wproj")
            (eng or nc.gpsimd).tensor_copy(t[:], raw[:])
            return t

        # ---- prologue: stage expert 0. HWDGE loads (x slab 0, both weights)
        # go first so they stream immediately; dependent stores/casts follow.
        # e0's casts use DVE (fast; no eviction copies exist yet to convoy).
        # Slabs 5-7 finish their chains during blocks 1-3.
        wfcs = [None] * E_PER
        wprojs = [None] * E_PER
        x_load(0, 0)
        rfc = load_wfc_raw(0)
        rpj = load_wproj_raw(0)
        x_cast(0, 0, eng=nc.vector)
        x_store(0, 0)
        wfcs[0] = cast_wfc(rfc, eng=nc.vector)
        wprojs[0] = cast_wproj(rpj, eng=nc.vector)
        issue_transpose(0, 0)
        for s in range(1, SLABS_PER_E):
            x_load(0, s)
            if s <= 4:
                x_cast(0, s, eng=nc.vector)
                x_store(0, s)
                issue_transpose(0, s)

        ht_all = {}
        pso_cur = None   # pso tiles of the chunk whose MM2 runs next block
        wraw = None

        for g in range(G_TOTAL + 1):
            e, i = divmod(g, CHUNKS_PER_E)
            ep, ip = divmod(g - 1, CHUNKS_PER_E)  # chunk drained this block

            # ---- staging ladder: expert en's slab s is staged relative to
            # block G0 = 8*(en-1): HWDGE load @ G0+s, GpSimd cast @ G0+s+1,
            # bf16 store @ G0+s+2, XBar transpose @ G0+s+4. Weight casts run
            # on GpSimd too: it is the ONLY queue allowed to carry long
            # waits — everything it produces has multi-block slack, so its
            # FIFO head-blocking can't convoy the copies (DVE) or the DMA
            # issue stream (Sync), both of which feed tight PSUM WARs.
            if g < G_TOTAL:
                # tail of expert 0's prologue staging (slabs 5-7)
                if e == 0 and 1 <= i <= 3:
                    x_cast(0, i + 4, eng=nc.vector)
                if e == 0 and 2 <= i <= 4:
                    x_store(0, i + 3)
                if e == 0 and 3 <= i <= 5:
                    issue_transpose(0, i + 2)
                for en in range(1, E_PER):
                    G0 = 8 * (en - 1)
                    if g == G0:
                        wraw = load_wfc_raw(en)
                    elif g == G0 + 2:
                        wfcs[en] = cast_wfc(wraw)
                    elif g == G0 + 4:
                        wraw = load_wproj_raw(en)
                    elif g == G0 + 6:
                        wprojs[en] = cast_wproj(wraw)
                    if 0 <= (g - G0) <= 7:
                        x_load(en, g - G0)
                    if 0 <= (g - G0 - 1) <= 7:
                        x_cast(en, g - G0 - 1)
                    if 0 <= (g - G0 - 2) <= 7:
                        x_store(en, g - G0 - 2)
                    if 0 <= (g - G0 - 4) <= 7:
                        issue_transpose(en, g - G0 - 4)

            # ---- interleaved PE stream per block g:
            #   [MM1(g,hp0)][MM2(g-1,s0)+copy][MM1(g,hp1)][MM2(g-1,s1)+copy]..
            # MM2 is s-major: each pso runs its full kk=0..7 accumulation
            # group quickly (start=True clears has_written BANK-wide, so one
            # bank may only ever host one in-flight group), and its eviction
            # copy is emitted immediately after, so the bank is free a full
            # block before chunk g reuses it.
            if g >= 1:
                ob = out_p.tile([P, NC_CHUNK // P, D], F32, tag="ob",
                                name=f"ob{g}")
            for hp in range(HGROUPS):
                if g < G_TOTAL:
                    psh = ps_h.tile([P, HPACK, NC_CHUNK], F32, tag="psh")
                    for j in range(HPACK):
                        hi = hp * HPACK + j
                        for k in range(KD):
                            nc.tensor.matmul(
                                psh[:, j, :],
                                wfcs[e][:, k, hi * P:(hi + 1) * P],
                                xts[e][k][i][:],
                                start=(k == 0),
                                stop=(k == KD - 1),
                            )
                    ht = ht_p.tile([P, HPACK, NC_CHUNK], BF16, tag="ht")
                    nc.scalar.activation(
                        ht[:], psh[:], mybir.ActivationFunctionType.Gelu
                    )
                    ht_all[(g, hp)] = ht
                if g >= 1:
                    s = hp
                    wp = wprojs[ep]
                    for kk in range(KH):
                        nc.tensor.matmul(
                            pso_cur[s][:],
                            ht_all[(g - 1, kk // HPACK)][
                                :, kk % HPACK, s * P:(s + 1) * P
                            ],
                            wp[:, kk, :],
                            start=(kk == 0),
                            stop=(kk == KH - 1),
                        )
                    nc.vector.tensor_copy(ob[:, s, :], pso_cur[s][:])

            # ---- store chunk g-1 ----
            if g >= 1:
                if g == G_TOTAL:
                    # final chunk: 4 pipelined 128-row stores shrink the tail
                    for s in range(NC_CHUNK // P):
                        rs = slice(ip * NC_CHUNK + s * P,
                                   ip * NC_CHUNK + (s + 1) * P)
                        nc.sync.dma_start(out[ep, rs, :], ob[:, s, :])
                else:
                    csl = slice(ip * NC_CHUNK, (ip + 1) * NC_CHUNK)
                    nc.sync.dma_start(
                        out[ep, csl, :].rearrange("(s p) d -> p s d", p=P),
                        ob[:],
                    )
                for hp in range(HGROUPS):
                    del ht_all[(g - 1, hp)]

            # ---- allocate chunk g's MM2 accumulators for next block ----
            if g < G_TOTAL:
                pso_cur = [
                    ps_o.tile([P, D], F32, tag="pso", name=f"pso{g}_{s}")
                    for s in range(NC_CHUNK // P)
                ]

    _fix_waits(nc)
    return nc


_CACHE = {}


def _get_nc():
    if "nc" not in _CACHE:
        _CACHE["nc"] = _build()
    return _CACHE["nc"]


def kernel(x, w_fc, w_proj, trace=False):
    assert x.shape == (E, CAP, D) and w_fc.shape == (E, D, H)
    assert w_proj.shape == (E, H, D)
    nc = _get_nc()
    x = np.ascontiguousarray(x, dtype=np.float32)
    w_fc = np.ascontiguousarray(w_fc, dtype=np.float32)
    w_proj = np.ascontiguousarray(w_proj, dtype=np.float32)
    in_maps = [
        {
            "x": x[i * E_PER:(i + 1) * E_PER],
            "w_fc": w_fc[i * E_PER:(i + 1) * E_PER],
            "w_proj": w_proj[i * E_PER:(i + 1) * E_PER],
        }
        for i in range(N_CORES)
    ]
    res = run_bass_kernel_spmd(nc, in_maps, list(range(N_CORES)), trace=trace)
    out = np.concatenate([r["out"] for r in res.results], axis=0)
    if trace:
        kernel.last_results = res
    return out


# revision 32
# speedup vs baseline: 1.2750x; 1.2750x over previous
"""Expert-parallel MoE MLP (ExpertMLP) Bass kernel for 8 Trainium2 NeuronCores.

Problem: x[32,4096,256] @ w_fc[32,256,1024] -> gelu(erf) -> @ w_proj[32,1024,256].

Sharding: expert-parallel, 4 experts per core, no cross-core communication.

Per-core dataflow (v2 — software-pipelined):

  * x[e] is staged to bf16 in DRAM slab-by-slab (1024 rows): HWDGE f32 load
    -> GpSimd cast -> HWDGE bf16 store -> XBar DMA-transpose into SBUF as
    xT[d, c] slabs. v2 used SWDGE DRAM->DRAM cast-DMAs here; their packet
    flood starved the HWDGE queue to ~1% bandwidth and stalled the PE 62us.
  * MM1: hT[h_tile, c] += w_fc[d,h_tile].T @ xT[d, c-chunk]; GELU (ACT) evicts
    PSUM->SBUF in bf16.
  * MM2: out[c_sub, d] += hT_slice.T @ w_proj[h, d], PSUM -> DVE copy -> DMA.
  * The PE instruction stream interleaves [MM2 of chunk g-1] with [MM1 of
    chunk g] so the GELU chain (5.3us per chunk on ACT) has a full chunk of
    slack and never stalls the PE (v1 lost ~2.2us per chunk to it).
  * ~24 dummy matmuls on a zeroed tile run at t~0 to open the HAM clock gate
    (PE idles at 1.2 GHz otherwise) while DMAs stage the first slabs.
  * Weights and x slabs for expert e+1 prefetch across expert e's chunks.

All matmul operands are bf16 (PSUM accumulation stays fp32); measured rel err
vs the f32 reference ~4e-3 (tolerance 2e-2).
"""

import numpy as np
from contextlib import ExitStack

import bass_rust as _br
import concourse.bass as bass
import concourse.tile as tile
from concourse import mybir
from concourse.bass_utils import run_bass_kernel_spmd

E, CAP, D, H = 32, 4096, 256, 1024
N_CORES = 8
E_PER = E // N_CORES  # 4 experts per core
P = 128
F32 = mybir.dt.float32
BF16 = mybir.dt.bfloat16

KD = D // P            # 2 k-tiles in MM1's contraction
KH = H // P            # 8 k-tiles in MM2's contraction
NC_CHUNK = 512         # capacity chunk per pipeline block
CHUNKS_PER_E = CAP // NC_CHUNK   # 8
SLAB = 1024            # x staging slab (two chunks per slab)
SLABS_PER_E = CAP // SLAB        # 4
HPACK = 2              # h_tiles packed per PSUM tile / GELU call
HGROUPS = H // P // HPACK        # 4
G_TOTAL = E_PER * CHUNKS_PER_E   # 32 pipeline blocks of real work


def _fix_waits(nc):
    """walrus here accepts only one sync wait per instruction; hoist excess
    waits onto standalone EventSemaphore instructions inserted before the
    offender (same engine => same sequencer order)."""
    for fn in nc.m.functions:
        for bb in fn.blocks:
            new = []
            changed = False
            for inst in bb.instructions:
                si = inst.sync_info
                if si is not None and len(si.on_wait) > 1:
                    waits = list(si.on_wait)
                    for w in waits[:-1]:
                        ev = mybir.InstEventSemaphore(
                            name=nc.get_next_instruction_name()
                        )
                        ev.engine = inst.engine
                        ev.sync_info = _br.SyncInfo(on_wait=[w], on_update=[])
                        nc.register_instruction(ev)
                        new.append(ev)
                    inst.sync_info = _br.SyncInfo(
                        on_wait=waits[-1:], on_update=list(si.on_update)
                    )
                    changed = True
                new.append(inst)
            if changed:
                bb.instructions = new


def _build():
    nc = bass.Bass(trn_type="TRN2", target_bir_lowering=False, debug=False)
    x = nc.dram_tensor("x", [E_PER, CAP, D], F32, kind="ExternalInput").ap()
    w_fc = nc.dram_tensor("w_fc", [E_PER, D, H], F32, kind="ExternalInput").ap()
    w_proj = nc.dram_tensor("w_proj", [E_PER, H, D], F32, kind="ExternalInput").ap()
    out = nc.dram_tensor("out", [E_PER, CAP, D], F32, kind="ExternalOutput").ap()
    # bf16 staging of x, one DRAM tensor per (expert, slab): DRAM dependency
    # tracking is tensor-granular, so each XBar transpose starts as soon as
    # its own slab's cast lands.
    xbf = [
        [
            nc.dram_tensor(f"xbf{e}_{s}", [SLAB, D], BF16).ap()
            for s in range(SLABS_PER_E)
        ]
        for e in range(E_PER)
    ]

    with tile.TileContext(nc) as tc, ExitStack() as ctx:
        xtp = ctx.enter_context(tc.tile_pool(name="xtp", bufs=2 * SLABS_PER_E * KD))
        xraw_p = ctx.enter_context(tc.tile_pool(name="xraw", bufs=3))
        xbfs_p = ctx.enter_context(tc.tile_pool(name="xbfs", bufs=3))
        wload = ctx.enter_context(tc.tile_pool(name="wload", bufs=2))
        wfc_p = ctx.enter_context(tc.tile_pool(name="wfc", bufs=2))
        wproj_p = ctx.enter_context(tc.tile_pool(name="wproj", bufs=2))
        ht_p = ctx.enter_context(tc.tile_pool(name="ht", bufs=2 * HGROUPS))
        out_p = ctx.enter_context(tc.tile_pool(name="outp", bufs=4))
        misc_p = ctx.enter_context(tc.tile_pool(name="misc", bufs=1))
        ps_h = ctx.enter_context(tc.tile_pool(name="ps_h", bufs=2, space="PSUM"))
        ps_o = ctx.enter_context(tc.tile_pool(name="ps_o", bufs=4, space="PSUM"))

        # ---- PE warmup: the HAM clock gate keeps the PE at 1.2 GHz until it
        # has seen ~3.4us of sustained activity. Dummy matmuls on a zeroed
        # tile warm it while the first slab casts/transposes are in flight.
        wu = misc_p.tile([P, P], BF16, tag="wu")
        nc.vector.memset(wu[:], 0.0)
        wps = ps_o.tile([P, D], F32, tag="pso")
        for _ in range(40):
            nc.tensor.matmul(wps[:, 0:P], wu[:], wu[:], start=True, stop=True)

        xts = [[[None] * SLABS_PER_E for _ in range(KD)] for _ in range(E_PER)]
        SLABW = SLAB // P * D  # 2048 elems/partition per slab tile
        stage = {}  # in-flight x staging tiles: ('r'|'b', e, s) -> tile

        def x_load(e, s):
            rs = slice(s * SLAB, (s + 1) * SLAB)
            raw = xraw_p.tile([P, SLABW], F32, tag="xr", name=f"xr{e}_{s}")
            nc.sync.dma_start(
                raw[:], x[e][rs].rearrange("(p s) d -> p (s d)", p=P)
            )
            stage[("r", e, s)] = raw

        def x_cast(e, s, eng=None):
            # steady-state casts run on the otherwise-idle GpSimd engine:
            # slow (~7us/slab vs 1.3 on DVE) but cadence is one per ~14us,
            # and its FIFO head-blocking can't convoy the copies (DVE) or
            # the DMA issue stream (Sync). Prologue casts use DVE for speed.
            bfv = xbfs_p.tile([P, SLABW], BF16, tag="xb", name=f"xb{e}_{s}")
            (eng or nc.gpsimd).tensor_copy(bfv[:], stage.pop(("r", e, s))[:])
            stage[("b", e, s)] = bfv

        def x_store(e, s):
            nc.sync.dma_start(
                xbf[e][s].rearrange("(p s) d -> p (s d)", p=P),
                stage.pop(("b", e, s))[:],
            )

        def issue_transpose(e, s):
            for k in range(KD):
                t = xtp.tile([P, SLAB], BF16, tag="xt", name=f"xt{e}_{k}_{s}")
                nc.sync.dma_start_transpose(t[:], xbf[e][s][:, k * P:(k + 1) * P])
                xts[e][k][s] = t

        def load_wfc_raw(e):
            raw = wload.tile([P, KD, H], F32, tag="wl")
            nc.sync.dma_start(raw[:], w_fc[e].rearrange("(k p) h -> p k h", p=P))
            return raw

        def cast_wfc(raw, eng=None):
            t = wfc_p.tile([P, KD, H], BF16, tag="wfc")
            (eng or nc.gpsimd).tensor_copy(t[:], raw[:])
            return t

        def load_wproj_raw(e):
            raw = wload.tile([P, KH, D], F32, tag="wl")
            nc.sync.dma_start(raw[:], w_proj[e].rearrange("(k p) d -> p k d", p=P))
            return raw

        def cast_wproj(raw, eng=None):
            t = wproj_p.tile([P, KH, D], BF16, tag="wproj")
            (eng or nc.gpsimd).tensor_copy(t[:], raw[:])
            return t

        # ---- prologue: stage expert 0. HWDGE loads (x slab 0, both weights)
        # go first so they stream immediately; dependent stores/casts follow.
        # e0's casts use DVE (fast; no eviction copies exist yet to convoy).
        # Slabs 5-7 finish their chains during blocks 1-3.
        wfcs = [None] * E_PER
        wprojs = [None] * E_PER
        x_load(0, 0)
        rfc = load_wfc_raw(0)
        rpj = load_wproj_raw(0)
        x_cast(0, 0, eng=nc.vector)
        x_store(0, 0)
        wfcs[0] = cast_wfc(rfc, eng=nc.vector)
        wprojs[0] = cast_wproj(rpj, eng=nc.vector)
        issue_transpose(0, 0)
        for s in range(1, SLABS_PER_E):
            x_load(0, s)
            x_cast(0, s, eng=nc.vector)
            x_store(0, s)
            issue_transpose(0, s)

        ht_all = {}
        pso_cur = None   # pso tiles of the chunk whose MM2 runs next block
        wraw = None

        for g in range(G_TOTAL + 1):
            e, i = divmod(g, CHUNKS_PER_E)
            ep, ip = divmod(g - 1, CHUNKS_PER_E)  # chunk drained this block

            # ---- staging ladder: expert en's slab s is staged relative to
            # block G0 = 8*(en-1): HWDGE load @ G0+s, GpSimd cast @ G0+s+1,
            # bf16 store @ G0+s+2, XBar transpose @ G0+s+4. Weight casts run
            # on GpSimd too: it is the ONLY queue allowed to carry long
            # waits — everything it produces has multi-block slack, so its
            # FIFO head-blocking can't convoy the copies (DVE) or the DMA
            # issue stream (Sync), both of which feed tight PSUM WARs.
            if g < G_TOTAL:
                for en in range(1, E_PER):
                    G0 = 8 * (en - 1)
                    if g == G0:
                        wraw = load_wfc_raw(en)
                    elif g == G0 + 2:
                        wfcs[en] = cast_wfc(wraw)
                    elif g == G0 + 4:
                        wraw = load_wproj_raw(en)
                    elif g == G0 + 6:
                        wprojs[en] = cast_wproj(wraw)
                    if (g - G0) in (0, 2, 4, 6):
                        x_load(en, (g - G0) // 2)
                    if (g - G0 - 1) in (0, 2, 4, 6):
                        x_cast(en, (g - G0 - 1) // 2)
                    if (g - G0 - 4) in (0, 2, 4, 6):
                        x_store(en, (g - G0 - 4) // 2)
                    if (g - G0 - 6) in (0, 2, 4, 6):
                        issue_transpose(en, (g - G0 - 6) // 2)

            # ---- interleaved PE stream per block g:
            #   [MM1(g,hp0)][MM2(g-1,s0)+copy][MM1(g,hp1)][MM2(g-1,s1)+copy]..
            # MM2 is s-major: each pso runs its full kk=0..7 accumulation
            # group quickly (start=True clears has_written BANK-wide, so one
            # bank may only ever host one in-flight group), and its eviction
            # copy is emitted immediately after, so the bank is free a full
            # block before chunk g reuses it.
            if g >= 1:
                ob = out_p.tile([P, NC_CHUNK // P, D], F32, tag="ob",
                                name=f"ob{g}")
            for hp in range(HGROUPS):
                if g < G_TOTAL:
                    psh = ps_h.tile([P, HPACK, NC_CHUNK], F32, tag="psh")
                    sidx, soff = i // 2, (i % 2) * NC_CHUNK
                    for j in range(HPACK):
                        hi = hp * HPACK + j
                        for k in range(KD):
                            nc.tensor.matmul(
                                psh[:, j, :],
                                wfcs[e][:, k, hi * P:(hi + 1) * P],
                                xts[e][k][sidx][:, soff:soff + NC_CHUNK],
                                start=(k == 0),
                                stop=(k == KD - 1),
                            )
                    ht = ht_p.tile([P, HPACK, NC_CHUNK], BF16, tag="ht")
                    nc.scalar.activation(
                        ht[:], psh[:], mybir.ActivationFunctionType.Gelu
                    )
                    ht_all[(g, hp)] = ht
                if g >= 1:
                    s = hp
                    wp = wprojs[ep]
                    for kk in range(KH):
                        nc.tensor.matmul(
                            pso_cur[s][:],
                            ht_all[(g - 1, kk // HPACK)][
                                :, kk % HPACK, s * P:(s + 1) * P
                            ],
                            wp[:, kk, :],
                            start=(kk == 0),
                            stop=(kk == KH - 1),
                        )
                    nc.vector.tensor_copy(ob[:, s, :], pso_cur[s][:])

            # ---- store chunk g-1 ----
            if g >= 1:
                if g == G_TOTAL:
                    # final chunk: 4 pipelined 128-row stores shrink the tail
                    for s in range(NC_CHUNK // P):
                        rs = slice(ip * NC_CHUNK + s * P,
                                   ip * NC_CHUNK + (s + 1) * P)
                        nc.sync.dma_start(out[ep, rs, :], ob[:, s, :])
                else:
                    csl = slice(ip * NC_CHUNK, (ip + 1) * NC_CHUNK)
                    nc.sync.dma_start(
                        out[ep, csl, :].rearrange("(s p) d -> p s d", p=P),
                        ob[:],
                    )
                for hp in range(HGROUPS):
                    del ht_all[(g - 1, hp)]

            # ---- allocate chunk g's MM2 accumulators for next block ----
            if g < G_TOTAL:
                pso_cur = [
                    ps_o.tile([P, D], F32, tag="pso", name=f"pso{g}_{s}")
                    for s in range(NC_CHUNK // P)
                ]

    _fix_waits(nc)
    return nc


_CACHE = {}


def _get_nc():
    if "nc" not in _CACHE:
        _CACHE["nc"] = _build()
    return _CACHE["nc"]


def kernel(x, w_fc, w_proj, trace=False):
    assert x.shape == (E, CAP, D) and w_fc.shape == (E, D, H)
    assert w_proj.shape == (E, H, D)
    nc = _get_nc()
    x = np.ascontiguousarray(x, dtype=np.float32)
    w_fc = np.ascontiguousarray(w_fc, dtype=np.float32)
    w_proj = np.ascontiguousarray(w_proj, dtype=np.float32)
    in_maps = [
        {
            "x": x[i * E_PER:(i + 1) * E_PER],
            "w_fc": w_fc[i * E_PER:(i + 1) * E_PER],
            "w_proj": w_proj[i * E_PER:(i + 1) * E_PER],
        }
        for i in range(N_CORES)
    ]
    res = run_bass_kernel_spmd(nc, in_maps, list(range(N_CORES)), trace=trace)
    out = np.concatenate([r["out"] for r in res.results], axis=0)
    if trace:
        kernel.last_results = res
    return out
